# revision 7
# baseline (speedup 1.0000x reference)
"""Location-dependent 3D conv (AsymConv) on 8 TRN2 NeuronCores.

Math (per output voxel):
    out[b, 0, x, y, z] = sum_{i,j,l in 0..2} Xp[b, x+i, y+j, z+l] * W[x, y, z, (i*3+j)*3+l]
with Xp = edge-padded X by 1 plane on each spatial side.

Strategy:
  - Shard the X spatial axis (96 = 8 cores x 12 planes). Host slices overlapping
    halo windows (14 planes) per core -> no inter-core communication at all.
  - Per core, SBUF layout: partition dim = y (96 used of 128), free = (b, x, z).
    Compute-engine APs must start at partition 0/32/64/96, so the y-shift cannot
    be a partition offset: the host ships 3 y-pre-shifted copies of the (small)
    X shard. The idle ScalarE then builds 9 (j, l) copies with the z-shift baked
    in and no z padding, so every Vector-engine read is a single contiguous
    4B-aligned run (fp16 2x perf mode, no per-run break overhead).
  - Products patch*W run on the Vector engine in fp16, one op per (tap, b).
  - The 27-term accumulation runs on the otherwise-idle TensorEngine as
    identity-matmuls accumulating into PSUM (fp32): 5 x 512-wide flat chunks
    per tap. A dummy matmul spin at kernel start warms the PE HAM clock gate
    during the DMA phase.
  - W (the 6 MB stream that dominates DMA) is shipped y-major in tap-issue
    order so each of its 7 chunk-DMAs moves multi-KB contiguous runs per
    partition (small per-row descriptors were capping DMA at ~160 GB/s).
  - PSUM -> SBUF (ScalarE) -> DRAM in fp32; host reassembles the full tensor.
"""

import os

import numpy as np

# ---- problem constants (hardcoded per harness rules) ----
B = 2
D = 96  # Dx = Dy = Dz
KSZ = 3
NTAP = KSZ**3  # 27
NCORES = 8
XS = D // NCORES  # 12 x-planes per core
XH = XS + 2  # with halo
ZP = D + 2  # padded z
NVOX = B * XS * D  # free elems per partition-row = 2304

F16 = np.float16
LAST_RESULT = None  # BassKernelResults of the most recent run (for test.py)

_GRAPH_CACHE = {}

N_WARMUP = int(os.environ.get("ASYM_WARMUP", "10"))

# taps with l != 1 first: their reads are aligned even in the padded base
# copies, which the first few taps use while ScalarE builds the contiguous ones
TAP_ORDER = [t for t in range(NTAP) if t % 3 != 1] + [t for t in range(NTAP) if t % 3 == 1]
N_PADDED_TAPS = 4  # first few taps read the padded base tiles directly

# W chunk sizes (in taps, along TAP_ORDER): first small so compute starts early
W_CHUNKS = [2, 3, 4, 4, 4, 5, 5]
assert sum(W_CHUNKS) == NTAP

# flat 512-wide PSUM chunks over the (b, x, z) = 2304 free dim
CHUNKS = [(c, min(512, NVOX - c)) for c in range(0, NVOX, 512)]


def _build_graph():
    """Build (and cache) the per-core Bass graph. Same graph for all 8 cores."""
    if "nc" in _GRAPH_CACHE:
        return _GRAPH_CACHE["nc"]

    from concourse import bacc
    import concourse.mybir as mybir
    from concourse.tile import TileContext

    f16 = mybir.dt.float16
    f32 = mybir.dt.float32

    nc = bacc.Bacc("TRN2", target_bir_lowering=False, debug=False, num_devices=NCORES)

    # y-pre-shifted X copies: xj[y', b, x, z] = Xp[y'+j, b, x, z]
    x_ds = [
        nc.dram_tensor(f"x{j}", [D, B, XH, ZP], f16, kind="ExternalInput")
        for j in range(KSZ)
    ]
    # [y, tap (TAP_ORDER-permuted), x, z] so chunk DMAs are per-partition contiguous
    w_d = nc.dram_tensor("w", [D, NTAP, XS, D], f16, kind="ExternalInput")
    id_d = nc.dram_tensor("ident", [D, D], f16, kind="ExternalInput")
    out_d = nc.dram_tensor("out", [D, B, XS, D], f32, kind="ExternalOutput")

    with TileContext(nc) as tc:
        with (
            tc.tile_pool(name="xp", bufs=1) as xpool,
            tc.tile_pool(name="wp", bufs=1) as wpool,
            tc.tile_pool(name="pp", bufs=1) as ppool,
            tc.tile_pool(name="psp", bufs=1, space="PSUM") as pspool,
        ):
            # identity first (tiny), then x0 and the first W chunk
            id_t = xpool.tile([D, D], f16, name="id_t", tag="id_t")
            nc.sync.dma_start(out=id_t[:], in_=id_d.ap())

            # PE warm-up: spin dummy matmuls during the DMA phase so the HAM
            # clock gate reaches 2.4 GHz before the real accumulation starts
            if N_WARMUP:
                dummy = ppool.tile([D, 512], f16, name="dummy", tag="warm_rhs", bufs=1)
                nc.vector.memset(dummy[:], 0.0)
                ps_w = pspool.tile([D, 512], f32, name="ps_warm", tag="ps_warm")
                for _ in range(N_WARMUP):
                    nc.tensor.matmul(ps_w[:], id_t[:], dummy[:], start=True, stop=True)

            x_ts = []
            for j in range(KSZ):
                xt = xpool.tile([D, B, XH, ZP], f16, name=f"x_{j}", tag=f"x_{j}")
                nc.sync.dma_start(out=xt[:], in_=x_ds[j].ap())
                x_ts.append(xt)
                if j == 0:
                    # first W chunk right after x0 so tap 0 can start
                    w_t = wpool.tile([D, NTAP, XS, D], f16, name="w_t", tag="w_t")
                    s0 = 0
                    nc.sync.dma_start(
                        out=w_t[:, s0 : s0 + W_CHUNKS[0]],
                        in_=w_d.ap()[:, s0 : s0 + W_CHUNKS[0]],
                    )
                    s0 += W_CHUNKS[0]
            for ntaps in W_CHUNKS[1:]:
                nc.sync.dma_start(
                    out=w_t[:, s0 : s0 + ntaps], in_=w_d.ap()[:, s0 : s0 + ntaps]
                )
                s0 += ntaps

            # contiguous unpadded (j, l) copies, built in first-use order
            xc = {}
            order = []
            for t in TAP_ORDER:
                jl = ((t // 3) % 3, t % 3)
                if jl not in order:
                    order.append(jl)
            for j, l in order:
                c = xpool.tile([D, B, XH, D], f16, name=f"xc_{j}_{l}", tag=f"xc_{j}_{l}")
                nc.scalar.copy(out=c[:], in_=x_ts[j][:, :, :, l : l + D])
                xc[(j, l)] = c

            psums = [
                pspool.tile([D, n], f32, name=f"ps_{ci}", tag=f"ps_{ci}")
                for ci, (c0, n) in enumerate(CHUNKS)
            ]
            for tn, t in enumerate(TAP_ORDER):
                i, j, l = t // 9, (t // 3) % 3, t % 3
                prod = ppool.tile([D, B, XS, D], f16, name="prod", tag="prod", bufs=6)
                for b in range(B):
                    if tn < N_PADDED_TAPS and l != 1:
                        in0 = x_ts[j][:, b : b + 1, i : i + XS, l : l + D]
                    else:
                        in0 = xc[(j, l)][:, b : b + 1, i : i + XS, :]
                    nc.vector.tensor_mul(
                        out=prod[:, b : b + 1],
                        in0=in0,
                        in1=w_t[:, tn : tn + 1, :, :],  # slot tn
                    )
                pflat = prod[:].rearrange("p a b c -> p (a b c)")
                for ci, (c0, n) in enumerate(CHUNKS):
                    nc.tensor.matmul(
                        psums[ci][:],
                        id_t[:],
                        pflat[:, c0 : c0 + n],
                        start=(tn == 0),
                        stop=(tn == NTAP - 1),
                    )
            oflat = out_d.ap().rearrange("p a b c -> p (a b c)")
            for ci, (c0, n) in enumerate(CHUNKS):
                outsb = ppool.tile([D, n], f32, name="outsb", tag=f"outsb_{ci}")
                nc.scalar.copy(out=outsb[:], in_=psums[ci][:])
                nc.sync.dma_start(out=oflat[:, c0 : c0 + n], in_=outsb[:])

    nc.compile()
    _GRAPH_CACHE["nc"] = nc
    return nc


def make_in_maps(X, W):
    """Host-side shard prep. X [2,1,96,96,96] f32, W [1,1,96,96,96,27] f32."""
    X = np.asarray(X)
    W = np.asarray(W)
    Xs = X.reshape(B, D, D, D)
    # edge padding on all three spatial dims
    Xp = np.pad(Xs, ((0, 0), (1, 1), (1, 1), (1, 1)), mode="edge")
    # -> [y, b, x, z]
    Xt = np.ascontiguousarray(np.transpose(Xp, (2, 0, 1, 3))).astype(F16)
    W00 = W.reshape(D, D, D, NTAP)
    ident = np.eye(D, dtype=F16)

    in_maps = []
    for m in range(NCORES):
        xs_full = Xt[:, :, m * XS : m * XS + XH, :]  # [98, 2, 14, 98]
        im = {"ident": ident}
        for j in range(KSZ):
            im[f"x{j}"] = np.ascontiguousarray(xs_full[j : j + D])
        wm = W00[m * XS : (m + 1) * XS]  # [12, 96, 96, 27]
        # [y, tap, x, z] with taps permuted into issue order
        wm = np.transpose(wm, (1, 3, 0, 2))[:, TAP_ORDER]
        im["w"] = np.ascontiguousarray(wm).astype(F16)
        in_maps.append(im)
    return in_maps


def kernel(X, W):
    global LAST_RESULT
    from concourse.bass_utils import run_bass_kernel_spmd

    nc = _build_graph()
    in_maps = make_in_maps(X, W)
    trace = bool(int(os.environ.get("ASYM_TRACE", "0")))
    res = run_bass_kernel_spmd(
        nc, in_maps, core_ids=list(range(NCORES)), trace=trace
    )
    LAST_RESULT = res

    out = np.empty((B, 1, D, D, D), dtype=np.float32)
    for m in range(NCORES):
        r = res.results[m]["out"]  # [y, b, x, z] f32
        out[:, 0, m * XS : (m + 1) * XS, :, :] = np.transpose(r, (1, 2, 0, 3))
    return out


# revision 8
# speedup vs baseline: 1.0770x; 1.0770x over previous
"""Location-dependent 3D conv (AsymConv) on 8 TRN2 NeuronCores.

Math (per output voxel):
    out[b, 0, x, y, z] = sum_{i,j,l in 0..2} Xp[b, x+i, y+j, z+l] * W[x, y, z, (i*3+j)*3+l]
with Xp = edge-padded X by 1 plane on each spatial side.

Strategy:
  - Shard the X spatial axis (96 = 8 cores x 12 planes). Host slices overlapping
    halo windows (14 planes) per core -> no inter-core communication at all.
  - Per core, SBUF layout: partition dim = y (96 used of 128), free = (b, x, z).
    Compute-engine APs must start at partition 0/32/64/96, so the y-shift cannot
    be a partition offset: the host ships 3 y-pre-shifted copies of the (small)
    X shard instead. The x/z shifts are plain free-dim AP offsets.
  - Products patch*W run on the Vector engine in fp16 (2x perf mode needs
    4-byte-aligned starts, so taps with l==1 read from a z-shifted copy made
    on the otherwise-idle ScalarE; those taps are issued last to hide the copies).
  - The 27-term accumulation runs on the otherwise-idle TensorEngine as
    identity-matmuls accumulating into PSUM (fp32), freeing the Vector engine
    from the adds. A dummy matmul spin at kernel start warms the PE HAM clock
    gate during the DMA phase.
  - W (the 6 MB stream that dominates DMA) is shipped y-major in tap-issue
    order so each of its 7 chunk-DMAs moves multi-KB contiguous runs per
    partition (small per-row descriptors were capping DMA at ~160 GB/s).
  - PSUM -> SBUF (ScalarE) -> DRAM in fp32; host reassembles the full tensor.
"""

import os

import numpy as np

# ---- problem constants (hardcoded per harness rules) ----
B = 2
D = 96  # Dx = Dy = Dz
KSZ = 3
NTAP = KSZ**3  # 27
NCORES = 8
XS = D // NCORES  # 12 x-planes per core
XH = XS + 2  # with halo
ZP = D + 2  # padded z

F16 = np.float16
LAST_RESULT = None  # BassKernelResults of the most recent run (for test.py)

_GRAPH_CACHE = {}

N_WARMUP = int(os.environ.get("ASYM_WARMUP", "10"))

# taps with l != 1 are 4B-aligned in the base copies; issue them first so the
# ScalarE z-shift copies (needed by l == 1 taps) are off the critical path
TAP_ORDER = [t for t in range(NTAP) if t % 3 != 1] + [t for t in range(NTAP) if t % 3 == 1]

# W chunk sizes (in taps, along TAP_ORDER): first small so compute starts early
W_CHUNKS = [2, 3, 4, 4, 4, 5, 5]
assert sum(W_CHUNKS) == NTAP

# per-b x-chunks whose fp32 free size fits one 2KB PSUM bank
CH = [(0, 5), (5, 5), (10, 2)]


def _build_graph():
    """Build (and cache) the per-core Bass graph. Same graph for all 8 cores."""
    if "nc" in _GRAPH_CACHE:
        return _GRAPH_CACHE["nc"]

    from concourse import bacc
    import concourse.mybir as mybir
    from concourse.tile import TileContext

    f16 = mybir.dt.float16
    f32 = mybir.dt.float32

    nc = bacc.Bacc("TRN2", target_bir_lowering=False, debug=False, num_devices=NCORES)

    # y-pre-shifted X copies: xj[y', b, x, z] = Xp[y'+j, b, x, z]
    x_ds = [
        nc.dram_tensor(f"x{j}", [D, B, XH, ZP], f16, kind="ExternalInput")
        for j in range(KSZ)
    ]
    # [y, tap (TAP_ORDER-permuted), x, z] so chunk DMAs are per-partition contiguous
    w_d = nc.dram_tensor("w", [D, NTAP, XS, D], f16, kind="ExternalInput")
    id_d = nc.dram_tensor("ident", [D, D], f16, kind="ExternalInput")
    out_d = nc.dram_tensor("out", [D, B, XS, D], f32, kind="ExternalOutput")

    with TileContext(nc) as tc:
        with (
            tc.tile_pool(name="xp", bufs=1) as xpool,
            tc.tile_pool(name="wp", bufs=1) as wpool,
            tc.tile_pool(name="pp", bufs=1) as ppool,
            tc.tile_pool(name="psp", bufs=1, space="PSUM") as pspool,
        ):
            # identity first (tiny), then x0 and the first W chunk
            id_t = xpool.tile([D, D], f16, name="id_t", tag="id_t")
            nc.sync.dma_start(out=id_t[:], in_=id_d.ap())

            # PE warm-up: spin dummy matmuls during the DMA phase so the HAM
            # clock gate reaches 2.4 GHz before the real accumulation starts
            if N_WARMUP:
                dummy = ppool.tile([D, 480], f16, name="dummy", tag="warm_rhs", bufs=1)
                nc.vector.memset(dummy[:], 0.0)
                ps_w = pspool.tile([D, 480], f32, name="ps_warm", tag="ps_warm")
                for _ in range(N_WARMUP):
                    nc.tensor.matmul(ps_w[:], id_t[:], dummy[:], start=True, stop=True)

            x_ts = []
            for j in range(KSZ):
                xt = xpool.tile([D, B, XH, ZP], f16, name=f"x_{j}", tag=f"x_{j}")
                nc.sync.dma_start(out=xt[:], in_=x_ds[j].ap())
                x_ts.append(xt)
                if j == 0:
                    # first W chunk right after x0 so tap 0 can start
                    w_t = wpool.tile([D, NTAP, XS, D], f16, name="w_t", tag="w_t")
                    s0 = 0
                    nc.sync.dma_start(
                        out=w_t[:, s0 : s0 + W_CHUNKS[0]],
                        in_=w_d.ap()[:, s0 : s0 + W_CHUNKS[0]],
                    )
                    s0 += W_CHUNKS[0]
            for ntaps in W_CHUNKS[1:]:
                nc.sync.dma_start(
                    out=w_t[:, s0 : s0 + ntaps], in_=w_d.ap()[:, s0 : s0 + ntaps]
                )
                s0 += ntaps

            x1_ts = []  # z-shifted by 1 (l = 1)
            for j in range(KSZ):
                x1 = xpool.tile([D, B, XH, ZP - 1], f16, name=f"xz_{j}", tag=f"xz_{j}")
                nc.scalar.copy(out=x1[:], in_=x_ts[j][:, :, :, 1:ZP])
                x1_ts.append(x1)

            for b in range(B):
                psums = [
                    pspool.tile([D, nx, D], f32, name=f"ps_{b}_{ci}", tag=f"ps_{b}_{ci}")
                    for ci, (x0, nx) in enumerate(CH)
                ]
                for tn, t in enumerate(TAP_ORDER):
                    i, j, l = t // 9, (t // 3) % 3, t % 3
                    src, le = (x_ts[j], l) if l != 1 else (x1_ts[j], 0)
                    prod = ppool.tile([D, XS, D], f16, name="prod", tag="prod", bufs=8)
                    nc.vector.tensor_mul(
                        out=prod[:],
                        in0=src[:, b, i : i + XS, le : le + D],
                        in1=w_t[:, tn, :, :],  # slot tn
                    )
                    for ci, (x0, nx) in enumerate(CH):
                        nc.tensor.matmul(
                            psums[ci][:],
                            id_t[:],
                            prod[:, x0 : x0 + nx, :],
                            start=(tn == 0),
                            stop=(tn == NTAP - 1),
                        )
                for ci, (x0, nx) in enumerate(CH):
                    outsb = ppool.tile([D, nx, D], f32, name="outsb", tag=f"outsb_{ci}")
                    nc.scalar.copy(out=outsb[:], in_=psums[ci][:])
                    nc.sync.dma_start(
                        out=out_d.ap()[:, b, x0 : x0 + nx, :],
                        in_=outsb[:],
                    )

    nc.compile()
    _GRAPH_CACHE["nc"] = nc
    return nc


def make_in_maps(X, W):
    """Host-side shard prep. X [2,1,96,96,96] f32, W [1,1,96,96,96,27] f32."""
    X = np.asarray(X)
    W = np.asarray(W)
    Xs = X.reshape(B, D, D, D)
    # edge padding on all three spatial dims
    Xp = np.pad(Xs, ((0, 0), (1, 1), (1, 1), (1, 1)), mode="edge")
    # -> [y, b, x, z]
    Xt = np.ascontiguousarray(np.transpose(Xp, (2, 0, 1, 3))).astype(F16)
    W00 = W.reshape(D, D, D, NTAP)
    ident = np.eye(D, dtype=F16)

    in_maps = []
    for m in range(NCORES):
        xs_full = Xt[:, :, m * XS : m * XS + XH, :]  # [98, 2, 14, 98]
        im = {"ident": ident}
        for j in range(KSZ):
            im[f"x{j}"] = np.ascontiguousarray(xs_full[j : j + D])
        wm = W00[m * XS : (m + 1) * XS]  # [12, 96, 96, 27]
        # [y, tap, x, z] with taps permuted into issue order
        wm = np.transpose(wm, (1, 3, 0, 2))[:, TAP_ORDER]
        im["w"] = np.ascontiguousarray(wm).astype(F16)
        in_maps.append(im)
    return in_maps


def kernel(X, W):
    global LAST_RESULT
    from concourse.bass_utils import run_bass_kernel_spmd

    nc = _build_graph()
    in_maps = make_in_maps(X, W)
    trace = bool(int(os.environ.get("ASYM_TRACE", "0")))
    res = run_bass_kernel_spmd(
        nc, in_maps, core_ids=list(range(NCORES)), trace=trace
    )
    LAST_RESULT = res

    out = np.empty((B, 1, D, D, D), dtype=np.float32)
    for m in range(NCORES):
        r = res.results[m]["out"]  # [y, b, x, z] f32
        out[:, 0, m * XS : (m + 1) * XS, :, :] = np.transpose(r, (1, 2, 0, 3))
    return out
